# revision 21
# baseline (speedup 1.0000x reference)
"""Trainium2 Bass kernel for nn_MixedLipMlp (soft-MoE MLP with Lipschitz gate).

Strategy: data-parallel over batch B=4096 across 8 NeuronCores (512 rows each,
expert weights + gate replicated). Activations live feature-major (features on
partitions, batch on the free dim) so layer GEMMs need no transposes; the
final tiny layer flips to batch-major so the coefficient mix is a per-
partition scalar op.

Everything the PE touches is fp16 (PSUM accumulation stays fp32): matmul cost
on TRN2 is free_size x 1 cycle/row for 16-bit operands, same as f32r, but
fp16 halves weight DMA, halves SBUF, and unlocks the DVE 2x mode for the
coefficient scalings. The Lipschitz row-scaling of the gate weights is folded
into the weights on the host (it depends only on inputs gw/gc).

DMA: per-queue bandwidth is only ~90GB/s, so transfers are split across the
sync/scalar/gpsimd queues in need-order: gate weights + inputs first, layer0
expert slabs interleaved across two queues so expert e lands before its
matmuls, layer1 slabs trailing.

Per-core schedule:
  gate:   hT = elu(gwS.T @ xT + gb) twice, logits, then softmax over the 8
          expert logits: exp (ACT) -> ones-matmul partition-sum broadcast to
          128 partitions (PE) -> reciprocal_approx_fast (DVE) -> coeffT.
  coeff broadcast: row e of coeffT is broadcast to all 128 partitions with a
          one-hot [8,128] matmul + ACT copy (bcE_e), feeding the per-expert
          input scalings s_e = x (.) bcE_e on the DVE (fp16 2x mode).
  moe l0/l1: out = sum_e (coeff_e (.) x) @ W_e accumulated in a single PSUM
          bank per m-tile; z-feature matmuls run row-paired (two experts in
          disjoint PE row groups); bias folds in as a K=8 matmul with
          rhs=coeffT. The last expert runs m-major with the bias matmuls
          interleaved so each bank closes (and its ELU starts) early.
          elu(y) = min(exp(y)-1, relu(y)): 2 ACT + 1 DVE min.
  l2:     batch-major: out[b, e*12+o] accumulates over K-tiles of [z;h1;1]
          (ones row carries the bias), all 8 experts packed on the free dim
          (N=96), K-major so each h1 m-tile unlocks work as its ELU ends;
          the mix sum_e coeff[b,e] * blk_e is 8 chained scalar_tensor_tensor
          ops with the per-partition coeff column (coeffB = tiny identity-
          matmul transpose of coeffT, done right after the softmax).
"""

import os
import sys

if "/opt/trn_rl_repo" not in sys.path:
    sys.path.insert(0, "/opt/trn_rl_repo")

# recover cleanly if a previous process left the NeuronCores wedged
os.environ.setdefault("NEURON_RT_RESET_CORES", "1")

import numpy as np

# Problem dimensions (hardcoded; must match the grader's setup_inputs()).
B = 4096
NCORES = 8
BS = B // NCORES  # 512 batch rows per core = matmul free dim
LATENT = 64
INPUT_SIZE = 256
IN_DIM = LATENT + INPUT_SIZE  # 320
HIDDEN = 512
ACTIONS = 12
E = 8
GATE_H = 128
INTER = HIDDEN + LATENT  # 576

NK0 = 2   # layer0: c has 256 rows = 2 k-slabs
NK12 = 4  # layers1,2: h has 512 rows = 4 k-slabs
NBT = BS // 128  # 4 batch tiles for the batch-major layer2

TRACE = False
LAST_EXEC_NS = None
LAST_RESULTS = None


def _build_nc():
    import concourse.mybir as mybir
    from concourse import bacc
    from concourse.tile import TileContext

    dt = mybir.dt
    F32 = dt.float32
    F16 = dt.float16
    AF = mybir.ActivationFunctionType
    OP = mybir.AluOpType

    nc = bacc.Bacc("TRN2", target_bir_lowering=False)

    # ---- DRAM I/O ------------------------------------------------------
    # gatepack cols: gw0a(0:128,rows<64) gw0b(128:256) gw0c(256:384)
    #   gw1(384:512) gw2(512:520); all Lipschitz-folded on the host
    d_gate = nc.dram_tensor("gatepack", [128, 520], F16, kind="ExternalInput")
    d_selp = nc.dram_tensor("selpack", [E, 1032], F16, kind="ExternalInput")
    d_gb = nc.dram_tensor("gbpack", [128, 3], F32, kind="ExternalInput")
    # per-core inputs: xinA = zdup(0:512) c0(512:1024);
    #                  xinB = c1(0:512) xzo(512:1024, rows<65)
    d_xinA = nc.dram_tensor("xinA", [128, 1024], F16, kind="ExternalInput")
    d_xinB = nc.dram_tensor("xinB", [128, 1024], F16, kind="ExternalInput")
    d_wz0 = nc.dram_tensor("wz0", [128, E // 2 * HIDDEN], F16,
                           kind="ExternalInput")
    # wz1 cols: w1z(0:2048) w2h(2048:2432) w2z(2432:2528, rows<65)
    d_wz1 = nc.dram_tensor("wz1", [128, 2528], F16, kind="ExternalInput")
    d_bp = nc.dram_tensor("bpack", [E, 2 * HIDDEN], F16, kind="ExternalInput")
    d_w0h = nc.dram_tensor("w0hcat", [128, E * NK0 * HIDDEN], F16,
                           kind="ExternalInput")
    d_w1h = nc.dram_tensor("w1hcat", [128, E * NK12 * HIDDEN], F16,
                           kind="ExternalInput")
    d_out = nc.dram_tensor("outB", [128, NBT, ACTIONS], F32,
                           kind="ExternalOutput")

    mm = nc.tensor.matmul

    with TileContext(nc) as tc:
        from contextlib import ExitStack

        with ExitStack() as ctx:
            pers = ctx.enter_context(tc.tile_pool(name="pers", bufs=1))
            sca = ctx.enter_context(tc.tile_pool(name="sca", bufs=10))
            etmp = ctx.enter_context(tc.tile_pool(name="etmp", bufs=4))

            # ---- DMA: 3 queues, need-ordered -----------------------------
            SL0 = NK0 * HIDDEN    # 1024 cols per l0 expert slab
            SL1 = NK12 * HIDDEN   # 2048 cols per l1 expert slab

            gate = pers.tile([128, 520], F16, tag="gate")
            nc.sync.dma_start(out=gate, in_=d_gate[:, :])
            selp = pers.tile([E, 1032], F16, tag="selp")
            nc.sync.dma_start(out=selp, in_=d_selp[:, :])
            xinA = pers.tile([128, 1024], F16, tag="xinA")
            nc.sync.dma_start(out=xinA, in_=d_xinA[:, :])

            gbp = pers.tile([128, 3], F32, tag="gbp")
            nc.scalar.dma_start(out=gbp, in_=d_gb[:, :])
            xinB = pers.tile([128, 1024], F16, tag="xinB")
            nc.scalar.dma_start(out=xinB, in_=d_xinB[:, :])

            w0hcat = pers.tile([128, E * SL0], F16, tag="w0hcat")
            w1hcat = pers.tile([128, E * SL1], F16, tag="w1hcat")
            # measured queue rates: sync ~165GB/s, scalar ~90GB/s, gpsimd
            # (software DGE) only ~40GB/s — gpsimd carries just 0.5MB.
            # sync: wz0 then l0 evens then l1 experts 0-3
            wz0 = pers.tile([128, E // 2 * HIDDEN], F16, tag="wz0")
            nc.sync.dma_start(out=wz0, in_=d_wz0[:, :])
            for e in [0, 2, 4, 6]:
                nc.sync.dma_start(out=w0hcat[:, e * SL0:(e + 1) * SL0],
                                  in_=d_w0h[:, e * SL0:(e + 1) * SL0])
            halfw1 = E * SL1 // 2
            nc.sync.dma_start(out=w1hcat[:, :halfw1], in_=d_w1h[:, :halfw1])
            # gpsimd: the two latest-needed l0 slabs only
            for e in [5, 7]:
                nc.gpsimd.dma_start(out=w0hcat[:, e * SL0:(e + 1) * SL0],
                                    in_=d_w0h[:, e * SL0:(e + 1) * SL0])
            # scalar: early l0 odds, then l1 z-slabs/bias, then l1 experts 4-7
            for e in [1, 3]:
                nc.scalar.dma_start(out=w0hcat[:, e * SL0:(e + 1) * SL0],
                                    in_=d_w0h[:, e * SL0:(e + 1) * SL0])
            wz1 = pers.tile([128, 2528], F16, tag="wz1")
            nc.scalar.dma_start(out=wz1, in_=d_wz1[:, :])
            bp = pers.tile([E, 2 * HIDDEN], F16, tag="bp")
            nc.scalar.dma_start(out=bp, in_=d_bp[:, :])
            nc.scalar.dma_start(out=w1hcat[:, halfw1:], in_=d_w1h[:, halfw1:])

            gw0t = [gate[0:64, 0:128], gate[:, 128:256], gate[:, 256:384]]
            gw1t = gate[:, 384:512]
            gw2t = gate[:, 512:520]
            sel8 = selp[:, 0:1024]
            eye8 = selp[:, 1024:1032]
            gb0 = gbp[:, 0:1]
            gb1 = gbp[:, 1:2]
            gb2 = gbp[0:E, 2:3]
            xz2 = xinA[:, 0:512]
            xc = [xinA[:, 512:1024], xinB[:, 0:512]]
            xzo = xinB[0:LATENT + 1, 512:1024]
            w0z = wz0
            w1z = wz1[:, 0:2048]
            w2h = wz1[:, 2048:2432]
            w2z = wz1[0:LATENT + 1, 2432:2528]
            b0sb = bp[:, 0:HIDDEN]
            b1sb = bp[:, HIDDEN:]
            w0h = [w0hcat[:, e * SL0:(e + 1) * SL0] for e in range(E)]
            w1h = [w1hcat[:, e * SL1:(e + 1) * SL1] for e in range(E)]

            # ---- constants -----------------------------------------------
            ones_blk = pers.tile([128, 128], F16, tag="ones_blk")
            nc.vector.memset(ones_blk, 1.0)
            warm_rhs = pers.tile([128, BS], F16, tag="warm_rhs")
            nc.vector.memset(warm_rhs, 0.0)

            def gate_elu(ps, bias, out_tag):
                # elu(y) = min(exp(y)-1, relu(y)); bias added in-op; exp on
                # ACT and relu on DVE run concurrently
                ex = etmp.tile([ps.shape[0], BS], F16, tag="elu_exp")
                nc.scalar.activation(out=ex, in_=ps, func=AF.Exp, bias=bias)
                rl = etmp.tile([ps.shape[0], BS], F16, tag="elu_relu")
                nc.vector.tensor_scalar(rl, ps, bias, 0.0, OP.add, OP.max)
                h = pers.tile([ps.shape[0], BS], F16, tag=out_tag)
                nc.vector.scalar_tensor_tensor(
                    out=h, in0=ex, scalar=1.0, in1=rl,
                    op0=OP.subtract, op1=OP.min,
                )
                return h

            def moe_elu(ps, out_tag):
                ex = etmp.tile([ps.shape[0], BS], F16, tag="elu_exp")
                nc.scalar.activation(out=ex, in_=ps, func=AF.Exp)
                rl = etmp.tile([ps.shape[0], BS], F16, tag="elu_relu")
                nc.scalar.activation(out=rl, in_=ps, func=AF.Relu)
                h = pers.tile([ps.shape[0], BS], F16, tag=out_tag)
                nc.vector.scalar_tensor_tensor(
                    out=h, in0=ex, scalar=1.0, in1=rl,
                    op0=OP.subtract, op1=OP.min,
                )
                return h

            # ---- gate + softmax + coefficient broadcasts -----------------
            # the whole chain is column-split into two 256-wide halves so
            # the serial latency (mm -> elu -> mm -> ... -> coeff) pipelines
            HB = BS // 2
            halves = [slice(0, HB), slice(HB, BS)]
            coeffB = []
            with tc.tile_pool(name="ps_g", bufs=2, space="PSUM") as ps_g, \
                 tc.tile_pool(name="ps_bc", bufs=2, space="PSUM") as ps_bc:

                # trip the PE activity monitor before the gate chain
                for _ in range(4):
                    pw = ps_bc.tile([128, BS], F32, tag="bc",
                                    name=f"warm{nc.next_id()}")
                    mm(pw, ones_blk, warm_rhs, start=True, stop=True)

                def gate_elu_h(ps, bias, out, sl, hi):
                    ex = etmp.tile([ps.shape[0], HB], F16, tag="elu_exp",
                                   name=f"gex{nc.next_id()}")
                    nc.scalar.activation(out=ex, in_=ps, func=AF.Exp, bias=bias)
                    rl = etmp.tile([ps.shape[0], HB], F16, tag="elu_relu",
                                   name=f"grl{nc.next_id()}")
                    nc.vector.tensor_scalar(rl, ps, bias, 0.0, OP.add, OP.max)
                    nc.vector.scalar_tensor_tensor(
                        out=out[:, sl], in0=ex, scalar=1.0, in1=rl,
                        op0=OP.subtract, op1=OP.min,
                    )

                h0g = pers.tile([GATE_H, BS], F16, tag="h0g")
                h1g = pers.tile([GATE_H, BS], F16, tag="h1g")
                expl = pers.tile([E, BS], F16, tag="expl")
                bcR = pers.tile([128, BS], F32, tag="bcR")
                coeffT = pers.tile([E, BS], F16, tag="coeffT")
                rhs0 = [xz2[:LATENT, :], xc[0], xc[1]]
                psg0, psg1, pslg, pssum = [], [], [], []
                for hi, sl in enumerate(halves):
                    p = ps_g.tile([GATE_H, HB], F32, tag="g", name=f"psg0{hi}")
                    for k in range(3):
                        mm(p, gw0t[k], rhs0[k][:, sl],
                           start=(k == 0), stop=(k == 2))
                    psg0.append(p)
                for hi, sl in enumerate(halves):
                    gate_elu_h(psg0[hi], gb0, h0g, sl, hi)
                for hi, sl in enumerate(halves):
                    p = ps_g.tile([GATE_H, HB], F32, tag="g", name=f"psg1{hi}")
                    mm(p, gw1t, h0g[:, sl], start=True, stop=True)
                    psg1.append(p)
                for hi, sl in enumerate(halves):
                    gate_elu_h(psg1[hi], gb1, h1g, sl, hi)
                for hi, sl in enumerate(halves):
                    p = ps_g.tile([E, HB], F32, tag="lg", name=f"pslg{hi}",
                                  bufs=2)
                    mm(p, gw2t, h1g[:, sl], start=True, stop=True)
                    pslg.append(p)
                # softmax over the 8 expert partitions (logits bounded by the
                # lip constraint, no max subtraction needed)
                for hi, sl in enumerate(halves):
                    nc.scalar.activation(out=expl[:, sl], in_=pslg[hi],
                                         func=AF.Exp, bias=gb2)
                    p = ps_bc.tile([128, HB], F32, tag="sum", name=f"pss{hi}",
                                   bufs=1)
                    mm(p, ones_blk[:E, :], expl[:, sl], start=True, stop=True)
                    pssum.append(p)
                for hi, sl in enumerate(halves):
                    nc.vector.reciprocal_approx_fast(out=bcR[:, sl],
                                                     in_=pssum[hi])
                    nc.vector.tensor_mul(coeffT[:, sl], expl[:, sl],
                                         bcR[:E, sl])

                # broadcast each normalized coeff row to all 128 partitions
                bcE = []
                for e in range(E):
                    pb = ps_bc.tile([128, BS], F32, tag="bc", name=f"pbc{e}",
                                    bufs=2)
                    mm(pb, sel8[:, 128 * e: 128 * (e + 1)], coeffT,
                       start=True, stop=True)
                    t = pers.tile([128, BS], F16, tag=f"bcE{e}")
                    nc.scalar.activation(out=t, in_=pb, func=AF.Copy)
                    bcE.append(t)

                # batch-major coeff for the l2 mix: tiny identity transposes
                for bt in range(NBT):
                    p8 = ps_bc.tile([128, E], F32, tag="bc", name=f"pc8_{bt}")
                    mm(p8, coeffT[:, 128 * bt: 128 * (bt + 1)], eye8,
                       start=True, stop=True)
                    t = pers.tile([128, E], F32, tag=f"cB{bt}",
                                  name=f"cB{bt}")
                    nc.scalar.activation(out=t, in_=p8, func=AF.Copy)
                    coeffB.append(t)

            # coeff-scaled z per expert (z duplicated in both 64-row halves
            # so an expert can ride either PE row group); shared by l0 and l1
            zsf = []
            for e in range(E):
                t = pers.tile([128, BS], F16, tag=f"zsf{e}")
                nc.vector.tensor_mul(t, xz2, bcE[e])
                zsf.append(t)
            # l0 scaled c inputs
            cs = []
            for e in range(E):
                for k in range(NK0):
                    t = sca.tile([128, BS], F16, tag="s", name=f"c{e}_{k}")
                    nc.vector.tensor_mul(t, xc[k], bcE[e])
                    cs.append(t)

            n_m = HIDDEN // 128  # 4

            def moe_layer(wz, wh, hs_tiles, nk, bsb, psl, htag):
                # pass A: row-paired z matmuls — two experts concurrently in
                # disjoint PE row groups. The top group only ever drains to
                # banks {0,1} and the bottom to {2,3}; the T1/T2 packing
                # swaps experts between groups so each covers all 4 m-slices.
                for p in range(E // 2):
                    for t_ in range(2):
                        base = p * HIDDEN + t_ * 256
                        etop = 2 * p + t_
                        ebot = 2 * p + 1 - t_
                        st = p == 0 and t_ == 0
                        for mi in range(2):
                            mm(psl[mi],
                               wz[:LATENT, base + 128 * mi: base + 128 * (mi + 1)],
                               zsf[etop][:LATENT, :],
                               start=st, stop=False)
                            mm(psl[2 + mi],
                               wz[LATENT:, base + 128 * mi: base + 128 * (mi + 1)],
                               zsf[ebot][LATENT:, :],
                               start=st, stop=False)
                # pass B: per-expert scaled h matmuls; the last expert runs
                # m-major with the bias close interleaved so each bank's ELU
                # can start while the remaining m-tiles still accumulate
                for e in range(E - 1):
                    for ki in range(nk):
                        for m in range(n_m):
                            mm(psl[m], wh[e][:, ki * HIDDEN + 128 * m:
                                             ki * HIDDEN + 128 * (m + 1)],
                               hs_tiles[e * nk + ki], start=False, stop=False)
                e = E - 1
                hts = []
                for m in range(n_m):
                    for ki in range(nk):
                        mm(psl[m], wh[e][:, ki * HIDDEN + 128 * m:
                                         ki * HIDDEN + 128 * (m + 1)],
                           hs_tiles[e * nk + ki], start=False, stop=False)
                    mm(psl[m], bsb[:, 128 * m: 128 * (m + 1)], coeffT,
                       start=False, stop=True)
                    hts.append(moe_elu(psl[m], f"{htag}{m}"))
                return hts

            # ---- MoE layers 0+1 share all 8 PSUM banks so layer 1's
            # z-pass can start while layer 0's ELU epilogue drains ---------
            acc_ctx = tc.tile_pool(name="ps_acc", bufs=8, space="PSUM")
            ps_acc = acc_ctx.__enter__()
            ps_l0 = [ps_acc.tile([128, BS], F32, tag="acc", name=f"psl0_{m}")
                     for m in range(n_m)]
            h0m = moe_layer(w0z, w0h, cs, NK0, b0sb, ps_l0, "h0m")

            # l1 scaled h inputs
            hs1 = []
            for e in range(E):
                for k in range(NK12):
                    t = sca.tile([128, BS], F16, tag="s", name=f"h{e}_{k}")
                    nc.vector.tensor_mul(t, h0m[k], bcE[e])
                    hs1.append(t)

            ps_l1 = [ps_acc.tile([128, BS], F32, tag="acc", name=f"psl1_{m}")
                     for m in range(n_m)]
            h1m = moe_layer(w1z, w1h, hs1, NK12, b1sb, ps_l1, "h1m")
            acc_ctx.__exit__(None, None, None)

            # ---- MoE layer 2, batch-major: out[b, e*12+o], N=96 ----------
            with tc.tile_pool(name="ps_l2", bufs=4, space="PSUM") as ps_l2:
                acc_all = pers.tile([128, NBT * ACTIONS], F32, tag="acc_all")
                ps96 = []
                for bt in range(NBT):
                    bsl = slice(128 * bt, 128 * (bt + 1))
                    p96 = ps_l2.tile([128, E * ACTIONS], F32, tag="l2",
                                     name=f"pl2_{bt}")
                    mm(p96, xzo[:, bsl], w2z, start=True, stop=False)
                    ps96.append(p96)
                # k-major so work unlocks as each h1m m-tile's ELU finishes;
                # the last k closes bt-by-bt with its mix chain right behind
                for k in range(NK12 - 1):
                    for bt in range(NBT):
                        bsl = slice(128 * bt, 128 * (bt + 1))
                        mm(ps96[bt], h1m[k][:, bsl],
                           w2h[:, k * E * ACTIONS: (k + 1) * E * ACTIONS],
                           start=False, stop=False)
                k = NK12 - 1
                for bt in range(NBT):
                    bsl = slice(128 * bt, 128 * (bt + 1))
                    mm(ps96[bt], h1m[k][:, bsl],
                       w2h[:, k * E * ACTIONS: (k + 1) * E * ACTIONS],
                       start=False, stop=True)
                    # mix: acc = sum_e coeffB[:, e] * blk_e (DVE; gpsimd
                    # can't read PSUM)
                    acc = acc_all[:, ACTIONS * bt: ACTIONS * (bt + 1)]
                    for e in range(E):
                        blk = ps96[bt][:, ACTIONS * e: ACTIONS * (e + 1)]
                        if e == 0:
                            nc.vector.tensor_scalar_mul(
                                acc, blk, coeffB[bt][:, 0:1])
                        else:
                            nc.vector.scalar_tensor_tensor(
                                out=acc, in0=blk,
                                scalar=coeffB[bt][:, e: e + 1], in1=acc,
                                op0=OP.mult, op1=OP.add,
                            )
                nc.sync.dma_start(out=d_out[:, :, :], in_=acc_all)

    nc.finalize()
    return nc


_nc_cache = None


def _get_nc():
    global _nc_cache
    if _nc_cache is None:
        _nc_cache = _build_nc()
    return _nc_cache


def _patch_hook_errors():
    # exceptions inside the neuronx-cc hook are swallowed by the PJRT
    # plugin ("CallFunctionObjArgs: error condition"); print them here
    from concourse import bass2jax

    orig = bass2jax.neuronx_cc_hook
    if getattr(orig, "_err_patched", False):
        return

    def wrapped(*a, **k):
        import traceback

        try:
            return orig(*a, **k)
        except BaseException as e:
            print(getattr(e, "output", ""), file=sys.stderr)
            traceback.print_exc()
            raise

    wrapped._err_patched = True
    bass2jax.neuronx_cc_hook = wrapped


def _pack_z_pairs(w):
    # (E, in, out) -> (128, E/2*out). For each expert pair p, two tiles of
    # (128, out/2): T1 = [top: even expert, first half of m-slices;
    # bottom: odd expert, second half], T2 = the swap — so the top PE row
    # group only ever produces the first half of output banks and the bottom
    # the second half, while both experts cover all output columns.
    z = w[:, :LATENT, :]
    out = z.shape[2]
    h = out // 2
    blk = np.empty((128, E // 2, 2, h), np.float32)
    for p in range(E // 2):
        blk[:LATENT, p, 0] = z[2 * p, :, :h]
        blk[LATENT:, p, 0] = z[2 * p + 1, :, h:]
        blk[:LATENT, p, 1] = z[2 * p + 1, :, :h]
        blk[LATENT:, p, 1] = z[2 * p, :, h:]
    return blk.reshape(128, -1)


def _lip_fold(gw, gc):
    # LipschitzLinear: rows of W scaled so row-wise L1 norm <= softplus(c);
    # depends only on the weights, so fold it on the host.
    lipc = np.logaddexp(0.0, np.float64(gc.reshape(())))
    scale = np.minimum(lipc / np.abs(np.float64(gw)).sum(1), 1.0)
    return (np.float64(gw) * scale[:, None]).astype(np.float32)


def _pack_weights(f):
    c = np.ascontiguousarray
    f16 = np.float16

    gate = np.zeros((128, 520), np.float32)
    gw0 = _lip_fold(f["gw0"], f["gc0"]).T  # [320, 128]
    gate[0:64, 0:128] = gw0[0:64]
    gate[:, 128:256] = gw0[64:192]
    gate[:, 256:384] = gw0[192:320]
    gate[:, 384:512] = _lip_fold(f["gw1"], f["gc1"]).T
    gate[:, 512:520] = _lip_fold(f["gw2"], f["gc2"]).T

    selp = np.zeros((E, 1032), np.float32)
    for e in range(E):
        selp[e, 128 * e: 128 * (e + 1)] = 1.0
    selp[:, 1024:1032] = np.eye(E)

    gbp = np.zeros((128, 3), np.float32)
    gbp[:, 0] = f["gb0"]
    gbp[:, 1] = f["gb1"]
    gbp[0:E, 2] = f["gb2"]

    w2 = f["w2"]  # (E, 576, 12)
    wz1 = np.zeros((128, 2528), np.float32)
    wz1[:, 0:2048] = _pack_z_pairs(f["w1"])
    wz1[:, 2048:2432] = (w2[:, LATENT:, :].reshape(E, NK12, 128, ACTIONS)
                         .transpose(2, 1, 0, 3).reshape(128, -1))
    wz1[0:LATENT, 2432:2528] = (w2[:, :LATENT, :].transpose(1, 0, 2)
                                .reshape(LATENT, -1))
    wz1[LATENT, 2432:2528] = f["b2"].reshape(-1)  # bias rides the ones row

    bpack = np.concatenate([f["b0"], f["b1"]], axis=1)  # [8, 1024]

    return {
        "gatepack": c(gate.astype(f16)),
        "selpack": c(selp.astype(f16)),
        "gbpack": c(gbp),
        "wz0": c(_pack_z_pairs(f["w0"]).astype(f16)),
        "wz1": c(wz1.astype(f16)),
        "bpack": c(bpack.astype(f16)),
        "w0hcat": c(f["w0"][:, LATENT:, :].reshape(E, NK0, 128, HIDDEN)
                    .transpose(2, 0, 1, 3).reshape(128, -1).astype(f16)),
        "w1hcat": c(f["w1"][:, LATENT:, :].reshape(E, NK12, 128, HIDDEN)
                    .transpose(2, 0, 1, 3).reshape(128, -1).astype(f16)),
    }


def kernel(**inputs):
    global LAST_EXEC_NS, LAST_RESULTS
    from concourse import bass_utils

    _patch_hook_errors()

    f = {k: np.ascontiguousarray(np.asarray(v, dtype=np.float32))
         for k, v in inputs.items()}

    shared = _pack_weights(f)
    in_maps = []
    for c in range(NCORES):
        sl = slice(c * BS, (c + 1) * BS)
        m = dict(shared)
        zT = f["z"][sl].T
        cT = f["c"][sl].T
        xa = np.zeros((128, 1024), np.float32)
        xa[0:LATENT, 0:512] = zT
        xa[LATENT:128, 0:512] = zT
        xa[:, 512:1024] = cT[0:128]
        xb = np.zeros((128, 1024), np.float32)
        xb[:, 0:512] = cT[128:256]
        xb[0:LATENT, 512:1024] = zT
        xb[LATENT, 512:1024] = 1.0  # ones row for the l2 bias
        m["xinA"] = np.ascontiguousarray(xa.astype(np.float16))
        m["xinB"] = np.ascontiguousarray(xb.astype(np.float16))
        in_maps.append(m)

    nc = _get_nc()
    res = bass_utils.run_bass_kernel_spmd(
        nc, in_maps, list(range(NCORES)), trace=TRACE
    )
    LAST_EXEC_NS = res.exec_time_ns
    LAST_RESULTS = res
    out = np.concatenate(
        [np.asarray(res.results[c]["outB"]).transpose(1, 0, 2).reshape(BS, ACTIONS)
         for c in range(NCORES)], axis=0
    )
    return out
